# revision 57
# baseline (speedup 1.0000x reference)
"""Deformable CNN block (offset conv -> deformable conv -> sync-BN -> ReLU)
as a Bass/Tile kernel for 8 Trainium2 NeuronCores, data-parallel over batch.

Per core (one batch item):
  - conv grid ("m-grid"): 114x114 padded grid, m=(ho+1)*114+(wo+1), padded
    to N=13312 = 13 groups x 1024. Packed plane layout: group g lives at
    partitions 32*(g//4)+k (k=tap), free block (g%4)*1024.
  - Within each 1024-group, gather SLOT j maps to position q = 64*(j%16) +
    j//16, so the idx table row r (= partitions the gather engine reads)
    is a CONTIGUOUS 128-B run of the q-order idx plane. Coef planes are
    written in slot order via permuted APs; the psy->ydef store un-permutes.
  - sample grid ("s-grid"): 118x118 zero-ringed image (image origin (2,2));
    a DRAM token table holds, per slot, the channel vectors of the four
    bilinear corners (t, t+1, t+118, t+119) = 256 bf16 = 512B.
  - per group: ONE dma_gather(transpose=True) of 9216 idx (9 taps x 1024)
    pulls [128=(64c|64c), 2, 9216] corner tiles; selector matmuls replicate
    compact bilinear coefs across partitions into PSUM, DVE forms the
    coef-weighted rhs, PE accumulates the deformable conv in PSUM.
  - sync-BN: per-channel sum/sumsq, AllReduce over 8 cores, fused
    scale+shift+ReLU on the scalar engine.
"""

import numpy as np
import ml_dtypes

import concourse.bass as bass
import concourse.bacc as bacc
import concourse.mybir as mybir
from concourse.bass_utils import run_bass_kernel_spmd
from concourse.tile import TileContext

F32 = mybir.dt.float32
BF16 = mybir.dt.bfloat16
I16 = mybir.dt.int16
BF = ml_dtypes.bfloat16

H = W = 112
C_IN, C_OUT, KK = 64, 128, 9
CG = 114
SG = 118
N = 13312
NG = CG * CG          # 12996
NGROUP, GSTR = 13, 1024
PBLK = 32
TOK = SG * SG         # 13924
TOKPAD = 13952
XC_OFF = 115
XCN = N + 2 * XC_OFF + 4
CHUNK = 512
NCHUNK = N // CHUNK
GSLICE = 1024
EPS = 1e-5
GIDX = KK * GSLICE    # 9216 idx per group gather

ADD = mybir.AluOpType.add
MULT = mybir.AluOpType.mult
SUB = mybir.AluOpType.subtract
MAXOP = mybir.AluOpType.max
MINOP = mybir.AluOpType.min


def _base_planes():
    m = np.arange(N)
    ry = (m // CG).astype(np.float32)
    rx = (m % CG).astype(np.float32)
    bY = np.zeros((128, 4 * GSTR), np.float32)
    bX = np.zeros((128, 4 * GSTR), np.float32)
    for g in range(NGROUP):
        sl = slice(g * GSTR, (g + 1) * GSTR)
        fb = slice((g % 4) * GSTR, (g % 4 + 1) * GSTR)
        for k in range(KK):
            p = PBLK * (g // 4) + k
            bY[p, fb] = ry[sl] + (k // 3)
            bX[p, fb] = rx[sl] + (k % 3)
    return bY, bX


def build_nc(w_off, b_off, w_dcn, b_dcn, gamma, beta, hw_loop=1, n_cores=8,
             no_cc=False, no_gather=False):
    nc = bacc.Bacc("TRN2", target_bir_lowering=False, num_devices=n_cores)

    x_in = nc.dram_tensor("x", [C_IN, H, W], F32, kind="ExternalInput")
    y_out = nc.dram_tensor("y", [C_OUT, H, W], F32, kind="ExternalOutput")

    # ---- host-prepacked constants ----
    w_off_r = w_off.reshape(KK, 2, C_IN, 3, 3)
    w_perm = np.concatenate([w_off_r[:, 0], w_off_r[:, 1]], 0)      # [18,64,3,3]
    b_perm = np.concatenate(
        [b_off.reshape(KK, 2)[:, 0], b_off.reshape(KK, 2)[:, 1]])   # [18]
    woff18 = np.stack(
        [w_perm[:, :, ky, kx].T for ky in range(3) for kx in range(3)], 1)
    woff_taps = np.zeros((C_IN, KK, 41), np.float32)
    woff_taps[:, :, 0:9] = woff18[:, :, 0:9]
    woff_taps[:, :, 32:41] = woff18[:, :, 9:18]
    # tap-paired offset-conv weights: rows 64-127 hold tap k+3 (the xcs
    # image copy in partitions 64-127 is pre-shifted by +CG rows)
    wpair_np = np.zeros((128, 3, 41), np.float32)
    wsing_np = np.zeros((C_IN, 3, 41), np.float32)
    for p in range(3):
        wpair_np[0:64, p] = woff_taps[:, p]
        wpair_np[64:128, p] = woff_taps[:, p + 3]
        wsing_np[:, p] = woff_taps[:, 6 + p]
    wpair_c = nc.inline_tensor(wpair_np.astype(BF), name="wpairT")
    wsing_c = nc.inline_tensor(wsing_np.astype(BF), name="wsingT")
    bY128 = np.zeros((128, 1), np.float32)
    bX128 = np.zeros((128, 1), np.float32)
    for b in range(4):
        bY128[PBLK * b:PBLK * b + 9, 0] = b_perm[0:9]
        bX128[PBLK * b:PBLK * b + 9, 0] = b_perm[9:18]
    boffY_c = nc.inline_tensor(bY128, name="boffY")
    boffX_c = nc.inline_tensor(bX128, name="boffX")
    bY_np, bX_np = _base_planes()
    bY_c = nc.inline_tensor(bY_np.astype(BF), name="baseY")
    bX_c = nc.inline_tensor(bX_np.astype(BF), name="baseX")
    wd = w_dcn.reshape(C_OUT, C_IN, 3, 3)
    wdup = np.stack(
        [np.concatenate([wd[:, :, k // 3, k % 3].T] * 2, 0) for k in range(KK)], 1)
    wdup_c = nc.inline_tensor(wdup.astype(BF), name="wdup")         # [128,9,128]
    bdcn_c = nc.inline_tensor(b_dcn.reshape(C_OUT, 1).astype(np.float32), name="bdcn")
    gam_c = nc.inline_tensor(gamma.reshape(C_OUT, 1).astype(np.float32), name="gam")
    bet_c = nc.inline_tensor(beta.reshape(C_OUT, 1).astype(np.float32), name="bet")
    id64_c = nc.inline_tensor(np.eye(64, dtype=BF), name="id64")
    mag1_c = nc.inline_tensor(np.full((128, 1), 8388607.5, np.float32), name="mag1")
    mag2_c = nc.inline_tensor(np.full((128, 1), -8388608.0, np.float32), name="mag2")
    # selector: lhsT slice b*9+k replicates packed row 32b+k to out rows
    # 0..63 and row 32b+16+k to out rows 64..127 (dual-corner planes)
    sel_np = np.zeros((128, 36, 128), np.float32)
    for b in range(4):
        for k in range(KK):
            sel_np[PBLK * b + k, b * 9 + k, 0:64] = 1.0
            sel_np[PBLK * b + 16 + k, b * 9 + k, 64:128] = 1.0
    sel_c = nc.inline_tensor(sel_np.astype(BF), name="sel2")

    tok_dram = nc.dram_tensor("tok", [TOKPAD, 256], BF16)
    # idx staging: gl2 = q-order rows [k][g][q]; gl3 = [g][r][k][c] so the
    # per-group wrap-16 idx table [16, 576] is a CONTIGUOUS block
    gl2_dram = nc.dram_tensor("gidxl2", [KK, 16, GSTR], I16)
    gl_dram = nc.dram_tensor("gidxl", [16, 16, KK, 64], I16)
    stats_in = nc.dram_tensor("statin", [C_OUT, 2], F32)
    stats_out = nc.dram_tensor("statout", [C_OUT, 2], F32, addr_space="Shared")

    with TileContext(nc) as tc:
        with (
            tc.tile_pool(name="big", bufs=1) as big,
            tc.tile_pool(name="work", bufs=2) as work,
            tc.tile_pool(name="psy", bufs=2, space="PSUM") as ppy,
        ):
            # dual-corner coef planes: allocated + zeroed once (outside the
            # timing loop) so never-written rows can't hold NaN garbage
            cT2 = big.tile([128, 4 * GSTR], BF16, tag="cT2")
            cB2 = big.tile([128, 4 * GSTR], BF16, tag="cB2")
            nc.vector.memset(cT2[:], 0.0)
            nc.vector.memset(cB2[:], 0.0)
            # py/px planes: persistent + zeroed once; per-iteration STT only
            # rewrites the used (row, col-block) regions
            pyP = big.tile([128, 4 * GSTR], F32, tag="pyP")
            pxP = big.tile([128, 4 * GSTR], F32, tag="pxP")
            nc.vector.memset(pyP[:], 0.0)
            nc.vector.memset(pxP[:], 0.0)

            loop = tc.For_i(0, hw_loop, 1) if hw_loop > 1 else None
            if loop is not None:
                loop.__enter__()

            # xc image: SBUF-resident for stages 1-5 (offset conv reads it
            # directly; no DRAM round-trip), freed before the gather stage
            sc15_cm = tc.tile_pool(name="sc15", bufs=1)
            sc15 = sc15_cm.__enter__()
            # rows 0-63: image at ring offset XC_OFF; rows 64-127: the same
            # image shifted +CG rows (tap-paired offset conv, K=128)
            xcs = sc15.tile([128, XCN], BF16, tag="xcs")
            nc.gpsimd.memset(xcs[:], 0.0)

            # ---- stage 1+2: bf16 grids; token table -> DRAM ----
            with (
                tc.tile_pool(name="sc12", bufs=1) as sc12,
                tc.tile_pool(name="ps12", bufs=4, space="PSUM") as ps12,
                tc.tile_pool(name="stgp", bufs=6) as stgp,
            ):
                xs = sc12.tile([64, TOK + 256], BF16, tag="xs")
                nc.gpsimd.memset(xs[:], 0.0)
                xs_img = xs[:, 0:TOK].rearrange(
                    "p (r c) -> p r c", r=SG)[:, 2:114, 2:114]
                nc.gpsimd.dma_start(out=xs_img, in_=x_in[:])
                xc_img = xcs[0:64, XC_OFF:XC_OFF + NG].rearrange(
                    "p (r c) -> p r c", r=CG)[:, 1:113, 1:113]
                nc.gpsimd.dma_start(out=xc_img, in_=x_in[:])
                xc_img2 = xcs[64:128, XC_OFF - CG:XC_OFF - CG + NG].rearrange(
                    "p (r c) -> p r c", r=CG)[:, 1:113, 1:113]
                nc.gpsimd.dma_start(out=xc_img2, in_=x_in[:])
                idt = sc12.tile([64, 64], BF16, tag="idt")
                nc.sync.dma_start(out=idt[:], in_=id64_c[:])
                qeng2 = (nc.sync, nc.scalar)
                TB = 4  # token blocks (128 tokens each) per copy+DMA
                nb = (TOK + 127) // 128  # 109
                for it, t0b in enumerate(range(0, nb, TB)):
                    nblk = min(TB, nb - t0b)
                    pst = ps12.tile([128, TB * 256], BF16, tag="pst")
                    for u in range(nblk):
                        j0 = (t0b + u) * 128
                        for di, dlt in enumerate((0, 1, SG, SG + 1)):
                            nc.tensor.transpose(
                                out=pst[:, u * 256 + di * 64:
                                        u * 256 + (di + 1) * 64],
                                in_=xs[:, j0 + dlt:j0 + dlt + 128],
                                identity=idt[:],
                            )
                    stg = stgp.tile([128, TB * 256], BF16, tag="stg")
                    if it % 2 == 0:
                        nc.scalar.copy(stg[:, 0:nblk * 256],
                                       pst[:, 0:nblk * 256])
                    else:
                        nc.vector.tensor_copy(stg[:, 0:nblk * 256],
                                              pst[:, 0:nblk * 256])
                    qeng2[it % 2].dma_start(
                        out=bass.AP(tok_dram, t0b * 128 * 256,
                                    [[256, 128], [128 * 256, nblk], [1, 256]]),
                        in_=stg[:, 0:nblk * 256])

            # ---- stages 3-5 ----
            # per-group idx tables: [128 part (8 replicas x 16 r), group, 576]
            gidx_w = big.tile([128, NGROUP, GIDX // 16], I16, tag="gidx_w")
            with (
                tc.tile_pool(name="sc34", bufs=1) as sc34,
                tc.tile_pool(name="ps34", bufs=2, space="PSUM") as ps34,
            ):
                scr = sc34.tile([128, 4 * GSTR], F32, tag="scr")
                wpT = sc34.tile([128, 3, 41], BF16, tag="wpT")
                wsT = sc34.tile([64, 3, 41], BF16, tag="wsT")
                boY = sc34.tile([128, 1], F32, tag="boY")
                boX = sc34.tile([128, 1], F32, tag="boX")
                bY = sc34.tile([128, 4 * GSTR], BF16, tag="bY")
                bX = sc34.tile([128, 4 * GSTR], BF16, tag="bX")
                mag1 = sc34.tile([128, 1], F32, tag="mag1")
                mag2 = sc34.tile([128, 1], F32, tag="mag2")
                nc.sync.dma_start(out=mag1[:], in_=mag1_c[:])
                nc.sync.dma_start(out=mag2[:], in_=mag2_c[:])
                nc.sync.dma_start(out=wpT[:], in_=wpair_c[:])
                nc.sync.dma_start(out=wsT[:], in_=wsing_c[:])
                nc.sync.dma_start(out=boY[:], in_=boffY_c[:])
                nc.sync.dma_start(out=boX[:], in_=boffX_c[:])
                nc.sync.dma_start(out=bY[:], in_=bY_c[:])
                nc.sync.dma_start(out=bX[:], in_=bX_c[:])
                for ch in range(NCHUNK):
                    n0 = ch * CHUNK
                    psc = ps34.tile([41, CHUNK], F32, tag="psc")
                    for p in range(3):  # taps (p, p+3) paired, K=128
                        dlt = -CG + (p - 1)
                        nc.tensor.matmul(
                            psc[:], wpT[:, p, :],
                            xcs[:, XC_OFF + dlt + n0:XC_OFF + dlt + n0 + CHUNK],
                            start=(p == 0), stop=False,
                        )
                    for q in range(3):  # taps 6,7,8 single, K=64
                        dlt = CG + (q - 1)
                        nc.tensor.matmul(
                            psc[:], wsT[:, q, :],
                            xcs[0:64, XC_OFF + dlt + n0:XC_OFF + dlt + n0 + CHUNK],
                            start=False, stop=(q == 2),
                        )
                    g, hf = ch // 2, ch % 2
                    rows = slice(PBLK * (g // 4), PBLK * (g // 4) + 9)
                    cols = slice((g % 4) * GSTR + hf * CHUNK,
                                 (g % 4) * GSTR + (hf + 1) * CHUNK)
                    nc.vector.scalar_tensor_tensor(
                        out=pyP[rows, cols], in0=psc[0:9, :], scalar=boY[rows],
                        in1=bY[rows, cols], op0=ADD, op1=ADD)
                    nc.vector.scalar_tensor_tensor(
                        out=pxP[rows, cols], in0=psc[32:41, :], scalar=boX[rows],
                        in1=bX[rows, cols], op0=ADD, op1=ADD)

                # indices + bilinear coefs (bY/bX freed into wy1/wx1 roles)
                y0b = sc34.tile([128, 4 * GSTR], BF16, tag="y0b")
                x0b = sc34.tile([128, 4 * GSTR], BF16, tag="x0b")
                wy1 = bY
                wx1 = bX
                nc.scalar.add(scr[:], pyP[:], mag1[:])
                nc.scalar.add(y0b[:], scr[:], mag2[:])
                nc.vector.tensor_tensor(wy1[:], pyP[:], y0b[:], SUB)
                nc.scalar.add(scr[:], pxP[:], mag1[:])
                nc.scalar.add(x0b[:], scr[:], mag2[:])
                nc.vector.tensor_tensor(wx1[:], pxP[:], x0b[:], SUB)
                nc.vector.tensor_scalar(y0b[:], y0b[:], 0.0, 115.0, MAXOP, MINOP)
                nc.vector.tensor_scalar(x0b[:], x0b[:], 0.0, 115.0, MAXOP, MINOP)
                gidx_f = pyP
                nc.vector.scalar_tensor_tensor(
                    out=gidx_f[:], in0=y0b[:], scalar=float(SG), in1=x0b[:],
                    op0=MULT, op1=ADD)
                # scr is dead from here on; reuse its storage for the i16 idx
                gidx_i = scr[:].bitcast(I16)[:, 0:4 * GSTR]
                nc.vector.tensor_copy(gidx_i, gidx_f[:])
                wy0 = y0b
                wx0 = x0b
                nc.vector.tensor_scalar(wy0[:], wy1[:], -1.0, 1.0, MULT, ADD)
                nc.vector.tensor_scalar(wx0[:], wx1[:], -1.0, 1.0, MULT, ADD)

                # corner products: q-order inputs -> slot(j)-order outputs.
                # TL/BL write their planes directly (junk in unused rows is
                # finite: py/px planes are zero there). TR/BR go through a
                # temp plane, then SBUF->SBUF DMA shifts rows 32b+k down to
                # 32b+16+k (engine ops can't start at partition 16, DMA can).
                cTt = sc34.tile([128, 4 * GSTR], BF16, tag="cTt")

                def jvF(t):
                    return t[:].rearrange("p (g c r) -> p g r c",
                                          g=4, c=64, r=16)

                def qvF(t):
                    return t[:].rearrange("p (g r c) -> p g r c",
                                          g=4, r=16, c=64)

                nc.vector.tensor_tensor(jvF(cT2), qvF(wy0), qvF(wx0), MULT)
                nc.vector.tensor_tensor(jvF(cTt), qvF(wy0), qvF(wx1), MULT)
                for b in range(4):
                    nc.scalar.dma_start(
                        out=cT2[PBLK * b + 16:PBLK * b + 25, :],
                        in_=cTt[PBLK * b:PBLK * b + 9, :])
                nc.vector.tensor_tensor(jvF(cB2), qvF(wy1), qvF(wx0), MULT)
                nc.vector.tensor_tensor(jvF(cTt), qvF(wy1), qvF(wx1), MULT)
                for b in range(4):
                    nc.scalar.dma_start(
                        out=cB2[PBLK * b + 16:PBLK * b + 25, :],
                        in_=cTt[PBLK * b:PBLK * b + 9, :])

                # idx staging: SBUF rows (q-order) -> gl2[k][g][q], then a
                # DRAM->DRAM shuffle per tap into gl3[g][r][k][c]
                GB = 16 * KK * 64  # 9216 elements per group block
                qeng = (nc.sync, nc.scalar)
                for k in range(KK):
                    for b in range(4):
                        row = PBLK * b + k
                        qeng[(k * 4 + b) % 2].dma_start(
                            out=bass.AP(gl2_dram, (k * 16 + 4 * b) * GSTR,
                                        [[1, 4 * GSTR]]),
                            in_=gidx_i[row:row + 1, :],
                        )
                for k in range(KK):
                    qeng[k % 2].dma_start(
                        out=bass.AP(gl_dram, k * 64,
                                    [[GB, 16], [KK * 64, 16], [1, 64]]),
                        in_=bass.AP(gl2_dram, k * 16 * GSTR,
                                    [[GSTR, 16], [1, GSTR]]),
                    )
                # idx tables: gidx_w[16a+r, s, 64k+c] = gl3[s][r][k][c], i.e.
                # slot j = 1024k + 16c + r samples position q = 64r + c.
                # One batched read per replica block a (all 13 groups).
                for a in range(8):
                    qeng[a % 2].dma_start(
                        out=gidx_w[16 * a:16 * (a + 1), :, :],
                        in_=bass.AP(gl_dram, 0,
                                    [[KK * 64, 16], [GB, NGROUP], [1, KK * 64]]),
                    )

            sc15_cm.__exit__(None, None, None)

            # ---- stage 6: deformable conv main loop ----
            ydef = big.tile([C_OUT, N], BF16, tag="ydef")
            bd = big.tile([C_OUT, 1], F32, tag="bd")
            nc.sync.dma_start(out=bd[:], in_=bdcn_c[:])
            wdupS = big.tile([128, KK, 128], BF16, tag="wdupS")
            nc.sync.dma_start(out=wdupS[:], in_=wdup_c[:])
            sel2 = big.tile([128, 36, 128], BF16, tag="sel2")
            nc.sync.dma_start(out=sel2[:], in_=sel_c[:])

            sc6_cm = tc.tile_pool(name="sc6", bufs=2)
            sc6 = sc6_cm.__enter__()
            ps6_cm = tc.tile_pool(name="ps6", bufs=2, space="PSUM")
            ps6 = ps6_cm.__enter__()
            for s in range(NGROUP):
                # two half-gathers: one 9216-idx gather overflows the SWDGE
                # ring. 4608 = 4.5 taps, and every (k, cc) 512-chunk lies
                # wholly in one half.
                hh = GIDX // 2
                gtA = sc6.tile([128, 2, hh], BF16, tag="gtA")
                gtB = sc6.tile([128, 2, hh], BF16, tag="gtB")
                gts = [gtA, gtB]
                for gh in range(2):
                    if no_gather:
                        nc.vector.memset(gts[gh][:], 0.0)
                    else:
                        nc.gpsimd.dma_gather(
                            out_ap=gts[gh][:], in_ap=tok_dram[:],
                            idxs_ap=gidx_w[:, s, gh * (hh // 16):(gh + 1) * (hh // 16)],
                            num_idxs=hh, num_idxs_reg=hh, elem_size=256,
                            transpose=True, single_packet=False,
                        )
                psyA = ppy.tile([C_OUT, CHUNK], F32, tag="psyA")
                psyB = ppy.tile([C_OUT, CHUNK], F32, tag="psyB")
                for k in range(KK):
                    for cc in range(GSLICE // CHUNK):  # 2
                        selk = sel2[:, (s // 4) * 9 + k, :]
                        cols = slice((s % 4) * GSTR + cc * CHUNK,
                                     (s % 4) * GSTR + (cc + 1) * CHUNK)
                        ctT = ps6.tile([128, CHUNK], F32, tag="ctT")
                        ctB = ps6.tile([128, CHUNK], F32, tag="ctB")
                        nc.tensor.matmul(ctT[:], selk,
                                         cT2[:, cols], start=True, stop=True)
                        nc.tensor.matmul(ctB[:], selk,
                                         cB2[:, cols], start=True, stop=True)
                        gT = work.tile([128, CHUNK], BF16, tag="gT")
                        gB = work.tile([128, CHUNK], BF16, tag="gB")
                        col = k * GSLICE + cc * CHUNK
                        gt = gts[col // hh]
                        gsl = slice(col % hh, col % hh + CHUNK)
                        if cc == 1:
                            # ACT-staged bf16 coefs -> DVE runs in 2x mode
                            stT = work.tile([128, CHUNK], BF16, tag="stT")
                            stB = work.tile([128, CHUNK], BF16, tag="stB")
                            nc.scalar.copy(stT[:], ctT[:])
                            nc.scalar.copy(stB[:], ctB[:])
                            nc.vector.tensor_tensor(
                                gT[:], gt[:, 0, gsl], stT[:], MULT)
                            nc.vector.tensor_tensor(
                                gB[:], gt[:, 1, gsl], stB[:], MULT)
                        else:
                            nc.vector.tensor_tensor(
                                gT[:], gt[:, 0, gsl], ctT[:], MULT)
                            nc.vector.tensor_tensor(
                                gB[:], gt[:, 1, gsl], ctB[:], MULT)
                        psy = psyA if cc == 0 else psyB
                        nc.tensor.matmul(psy[:], wdupS[:, k, :], gT[:],
                                         start=(k == 0), stop=False,
                                         skip_group_check=True)
                        nc.tensor.matmul(psy[:], wdupS[:, k, :], gB[:],
                                         start=False, stop=(k == KK - 1),
                                         skip_group_check=True)
                for cc in range(GSLICE // CHUNK):
                    psy = psyA if cc == 0 else psyB
                    # un-permute: psy col 16c+r -> ydef col 64r+c (+32cc, +1024s)
                    yv_blk = ydef[:, s * GSLICE:(s + 1) * GSLICE].rearrange(
                        "p (r c) -> p c r", r=16, c=64)[:, 32 * cc:32 * (cc + 1), :]
                    nc.scalar.add(
                        yv_blk,
                        psy[:].rearrange("p (c r) -> p c r", c=32, r=16),
                        bd[:])
            ps6_cm.__exit__(None, None, None)
            sc6_cm.__exit__(None, None, None)

            # ---- stage 7: BN stats + AllReduce ----
            ssum = big.tile([C_OUT, 8], F32, tag="ssum")
            ssq = big.tile([C_OUT, 8], F32, tag="ssq")
            yv = ydef[:, 0:NG].rearrange("p (r c) -> p r c", r=CG)
            for r in range(7):
                vap = yv[:, 1 + r * 16:1 + (r + 1) * 16, 1:113]
                nc.vector.tensor_reduce(
                    ssum[:, r:r + 1], vap, axis=mybir.AxisListType.XY, op=ADD)
                sqscr = work.tile([C_OUT, 16 * W], F32, tag="ofin")
                nc.vector.scalar_tensor_tensor(
                    out=sqscr[:].rearrange("p (a b) -> p a b", a=16), in0=vap,
                    scalar=1.0, in1=vap, op0=MULT, op1=MULT,
                    accum_out=ssq[:, r:r + 1])
            st2 = big.tile([C_OUT, 2], F32, tag="st2")
            nc.vector.tensor_reduce(
                st2[:, 0:1], ssum[:, 0:7], axis=mybir.AxisListType.X, op=ADD)
            nc.vector.tensor_reduce(
                st2[:, 1:2], ssq[:, 0:7], axis=mybir.AxisListType.X, op=ADD)
            nc.sync.dma_start(out=stats_in[:], in_=st2[:])
            if no_cc:
                nc.sync.dma_start(out=stats_out[:], in_=stats_in[:])
            else:
                nc.gpsimd.collective_compute(
                    "AllReduce", ADD, replica_groups=[list(range(n_cores))],
                    ins=[stats_in[:]], outs=[stats_out[:]])
            stg2 = big.tile([C_OUT, 2], F32, tag="stg2")
            nc.sync.dma_start(out=stg2[:], in_=stats_out[:])

            # ---- stage 8: BN affine + ReLU + store ----
            gam = big.tile([C_OUT, 1], F32, tag="gamt")
            bet = big.tile([C_OUT, 1], F32, tag="bett")
            nc.sync.dma_start(out=gam[:], in_=gam_c[:])
            nc.sync.dma_start(out=bet[:], in_=bet_c[:])
            NTOT = float(n_cores * H * W)
            mean = big.tile([C_OUT, 1], F32, tag="mean")
            var = big.tile([C_OUT, 1], F32, tag="var")
            nc.vector.tensor_scalar(mean[:], stg2[:, 0:1], 1.0 / NTOT, None, MULT)
            nc.vector.tensor_scalar(var[:], stg2[:, 1:2], 1.0 / NTOT, None, MULT)
            m2 = big.tile([C_OUT, 1], F32, tag="m2")
            nc.vector.tensor_tensor(m2[:], mean[:], mean[:], MULT)
            nc.vector.tensor_tensor(var[:], var[:], m2[:], SUB)
            nc.vector.tensor_scalar(var[:], var[:], EPS, None, ADD)
            sd = big.tile([C_OUT, 1], F32, tag="sd")
            nc.scalar.activation(sd[:], var[:], mybir.ActivationFunctionType.Sqrt)
            rsd = big.tile([C_OUT, 1], F32, tag="rsd")
            nc.vector.reciprocal(rsd[:], sd[:])
            aa = big.tile([C_OUT, 1], F32, tag="aa")
            bb2 = big.tile([C_OUT, 1], F32, tag="bb2")
            nc.vector.tensor_tensor(aa[:], gam[:], rsd[:], MULT)
            nc.vector.tensor_tensor(bb2[:], aa[:], mean[:], MULT)
            nc.vector.tensor_tensor(bb2[:], bet[:], bb2[:], SUB)
            for r in range(7):
                vap = yv[:, 1 + r * 16:1 + (r + 1) * 16, 1:113]
                ofin = work.tile([C_OUT, 16 * W], F32, tag="ofin")
                nc.scalar.activation(
                    ofin[:].rearrange("p (a b) -> p a b", a=16), vap,
                    mybir.ActivationFunctionType.Relu, bias=bb2[:], scale=aa[:])
                nc.sync.dma_start(
                    out=y_out[:, r * 16:(r + 1) * 16, :],
                    in_=ofin[:].rearrange("p (a b) -> p a b", a=16))

            if loop is not None:
                loop.__exit__(None, None, None)

    nc.compile()
    return nc


def kernel(x, w_off, b_off, w_dcn, b_dcn, gamma, beta):
    x = np.asarray(x, np.float32)
    nc = build_nc(
        np.asarray(w_off, np.float32), np.asarray(b_off, np.float32),
        np.asarray(w_dcn, np.float32), np.asarray(b_dcn, np.float32),
        np.asarray(gamma, np.float32), np.asarray(beta, np.float32),
    )
    in_maps = [{"x": np.ascontiguousarray(x[b])} for b in range(8)]
    res = run_bass_kernel_spmd(nc, in_maps, list(range(8)))
    return np.stack([res.results[b]["y"] for b in range(8)], 0).astype(np.float32)


# revision 68
# speedup vs baseline: 1.4135x; 1.4135x over previous
"""Deformable CNN block (offset conv -> deformable conv -> sync-BN -> ReLU)
as a Bass/Tile kernel for 8 Trainium2 NeuronCores, data-parallel over batch.

Per core (one batch item):
  - conv grid ("m-grid"): 114x114 padded grid, m=(ho+1)*114+(wo+1), padded
    to N=13312 = 13 groups x 1024. Packed plane layout: group g lives at
    partitions 32*(g//4)+k (k=tap), free block (g%4)*1024.
  - Within each 1024-group, gather SLOT j maps to position q = 64*(j%16) +
    j//16, so the idx table row r (= partitions the gather engine reads)
    is a CONTIGUOUS 128-B run of the q-order idx plane. Coef planes are
    written in slot order via permuted APs; the psy->ydef store un-permutes.
  - sample grid ("s-grid"): 118x118 zero-ringed image (image origin (2,2));
    a DRAM token table holds, per slot, the channel vectors of the four
    bilinear corners (t, t+1, t+118, t+119) = 256 bf16 = 512B.
  - per group: ONE dma_gather(transpose=True) of 9216 idx (9 taps x 1024)
    pulls [128=(64c|64c), 2, 9216] corner tiles; selector matmuls replicate
    compact bilinear coefs across partitions into PSUM, DVE forms the
    coef-weighted rhs, PE accumulates the deformable conv in PSUM.
  - sync-BN: per-channel sum/sumsq, AllReduce over 8 cores, fused
    scale+shift+ReLU on the scalar engine.
"""

import numpy as np
import ml_dtypes

import concourse.bass as bass
import concourse.bacc as bacc
import concourse.mybir as mybir
from concourse.bass_utils import run_bass_kernel_spmd
from concourse.tile import TileContext

F32 = mybir.dt.float32
BF16 = mybir.dt.bfloat16
I16 = mybir.dt.int16
BF = ml_dtypes.bfloat16

H = W = 112
C_IN, C_OUT, KK = 64, 128, 9
CG = 114
SG = 118
N = 13312
NG = CG * CG          # 12996
NGROUP, GSTR = 13, 1024
PBLK = 32
TOK = SG * SG         # 13924
TOKPAD = 13952
XC_OFF = 115
XCN = N + 2 * XC_OFF + 4
CHUNK = 512
NCHUNK = N // CHUNK
GSLICE = 1024
EPS = 1e-5
GIDX = KK * GSLICE    # 9216 idx per group gather

ADD = mybir.AluOpType.add
MULT = mybir.AluOpType.mult
SUB = mybir.AluOpType.subtract
MAXOP = mybir.AluOpType.max
MINOP = mybir.AluOpType.min


def _base_planes():
    m = np.arange(N)
    ry = (m // CG).astype(np.float32)
    rx = (m % CG).astype(np.float32)
    bY = np.zeros((128, 4 * GSTR), np.float32)
    bX = np.zeros((128, 4 * GSTR), np.float32)
    for g in range(NGROUP):
        sl = slice(g * GSTR, (g + 1) * GSTR)
        fb = slice((g % 4) * GSTR, (g % 4 + 1) * GSTR)
        for k in range(KK):
            p = PBLK * (g // 4) + k
            bY[p, fb] = ry[sl] + (k // 3)
            bX[p, fb] = rx[sl] + (k % 3)
    return bY, bX


def build_nc(w_off, b_off, w_dcn, b_dcn, gamma, beta, hw_loop=1, n_cores=8,
             no_cc=False, no_gather=False, act_stage=False, gather_sp=False,
             gather_notr=False, pe_transpose=False):
    nc = bacc.Bacc("TRN2", target_bir_lowering=False, num_devices=n_cores)

    x_in = nc.dram_tensor("x", [C_IN, H, W], F32, kind="ExternalInput")
    y_out = nc.dram_tensor("y", [C_OUT, H, W], F32, kind="ExternalOutput")

    # ---- host-prepacked constants ----
    w_off_r = w_off.reshape(KK, 2, C_IN, 3, 3)
    w_perm = np.concatenate([w_off_r[:, 0], w_off_r[:, 1]], 0)      # [18,64,3,3]
    b_perm = np.concatenate(
        [b_off.reshape(KK, 2)[:, 0], b_off.reshape(KK, 2)[:, 1]])   # [18]
    woff18 = np.stack(
        [w_perm[:, :, ky, kx].T for ky in range(3) for kx in range(3)], 1)
    woff_taps = np.zeros((C_IN, KK, 41), np.float32)
    woff_taps[:, :, 0:9] = woff18[:, :, 0:9]
    woff_taps[:, :, 32:41] = woff18[:, :, 9:18]
    # tap-paired offset-conv weights: rows 64-127 hold tap k+3 (the xcs
    # image copy in partitions 64-127 is pre-shifted by +CG rows)
    wpair_np = np.zeros((128, 3, 41), np.float32)
    wsing_np = np.zeros((C_IN, 3, 41), np.float32)
    for p in range(3):
        wpair_np[0:64, p] = woff_taps[:, p]
        wpair_np[64:128, p] = woff_taps[:, p + 3]
        wsing_np[:, p] = woff_taps[:, 6 + p]
    wpair_c = nc.inline_tensor(wpair_np.astype(BF), name="wpairT")
    wsing_c = nc.inline_tensor(wsing_np.astype(BF), name="wsingT")
    bY128 = np.zeros((128, 1), np.float32)
    bX128 = np.zeros((128, 1), np.float32)
    for b in range(4):
        bY128[PBLK * b:PBLK * b + 9, 0] = b_perm[0:9]
        bX128[PBLK * b:PBLK * b + 9, 0] = b_perm[9:18]
    boffY_c = nc.inline_tensor(bY128, name="boffY")
    boffX_c = nc.inline_tensor(bX128, name="boffX")
    bY_np, bX_np = _base_planes()
    bY_c = nc.inline_tensor(bY_np.astype(BF), name="baseY")
    bX_c = nc.inline_tensor(bX_np.astype(BF), name="baseX")
    wd = w_dcn.reshape(C_OUT, C_IN, 3, 3)
    wdup = np.stack(
        [np.concatenate([wd[:, :, k // 3, k % 3].T] * 2, 0) for k in range(KK)], 1)
    wdup_c = nc.inline_tensor(wdup.astype(BF), name="wdup")         # [128,9,128]
    bdcn_c = nc.inline_tensor(b_dcn.reshape(C_OUT, 1).astype(np.float32), name="bdcn")
    gam_c = nc.inline_tensor(gamma.reshape(C_OUT, 1).astype(np.float32), name="gam")
    bet_c = nc.inline_tensor(beta.reshape(C_OUT, 1).astype(np.float32), name="bet")
    id64_c = nc.inline_tensor(np.eye(64, dtype=BF), name="id64")
    id128_c = nc.inline_tensor(np.eye(128, dtype=BF), name="id128")
    mag1_c = nc.inline_tensor(np.full((128, 1), 8388607.5, np.float32), name="mag1")
    mag2_c = nc.inline_tensor(np.full((128, 1), -8388608.0, np.float32), name="mag2")
    # selector: lhsT slice b*9+k replicates packed row 32b+k to out rows
    # 0..63 and row 32b+16+k to out rows 64..127 (dual-corner planes)
    sel_np = np.zeros((128, 36, 128), np.float32)
    for b in range(4):
        for k in range(KK):
            sel_np[PBLK * b + k, b * 9 + k, 0:64] = 1.0
            sel_np[PBLK * b + 16 + k, b * 9 + k, 64:128] = 1.0
    sel_c = nc.inline_tensor(sel_np.astype(BF), name="sel2")

    tok_dram = nc.dram_tensor("tok", [TOKPAD, 256], BF16)
    # idx staging: gl2 = q-order rows [k][g][q]; gl3 = [g][r][k][c] so the
    # per-group wrap-16 idx table [16, 576] is a CONTIGUOUS block
    gl2_dram = nc.dram_tensor("gidxl2", [KK, 16, GSTR], I16)
    gl_dram = nc.dram_tensor("gidxl", [16, 16, KK, 64], I16)
    stats_in = nc.dram_tensor("statin", [C_OUT, 2], F32)
    stats_out = nc.dram_tensor("statout", [C_OUT, 2], F32, addr_space="Shared")

    with TileContext(nc) as tc:
        with (
            tc.tile_pool(name="big", bufs=1) as big,
            tc.tile_pool(name="work", bufs=2) as work,
            tc.tile_pool(name="psy", bufs=2, space="PSUM") as ppy,
        ):
            # dual-corner coef planes: allocated + zeroed once (outside the
            # timing loop) so never-written rows can't hold NaN garbage
            cT2 = big.tile([128, 4 * GSTR], BF16, tag="cT2")
            cB2 = big.tile([128, 4 * GSTR], BF16, tag="cB2")
            nc.vector.memset(cT2[:], 0.0)
            nc.vector.memset(cB2[:], 0.0)
            # py/px planes: persistent + zeroed once; per-iteration STT only
            # rewrites the used (row, col-block) regions
            pyP = big.tile([128, 4 * GSTR], F32, tag="pyP")
            pxP = big.tile([128, 4 * GSTR], F32, tag="pxP")
            nc.vector.memset(pyP[:], 0.0)
            nc.vector.memset(pxP[:], 0.0)

            loop = tc.For_i(0, hw_loop, 1) if hw_loop > 1 else None
            if loop is not None:
                loop.__enter__()

            # xc image: SBUF-resident for stages 1-5 (offset conv reads it
            # directly; no DRAM round-trip), freed before the gather stage
            sc15_cm = tc.tile_pool(name="sc15", bufs=1)
            sc15 = sc15_cm.__enter__()
            # rows 0-63: image at ring offset XC_OFF; rows 64-127: the same
            # image shifted +CG rows (tap-paired offset conv, K=128)
            xcs = sc15.tile([128, XCN], BF16, tag="xcs")
            nc.gpsimd.memset(xcs[:], 0.0)

            # ---- stage 1+2: bf16 grids; token table -> DRAM ----
            with (
                tc.tile_pool(name="sc12", bufs=1) as sc12,
                tc.tile_pool(name="ps12", bufs=4, space="PSUM") as ps12,
                tc.tile_pool(name="stgp", bufs=6) as stgp,
            ):
                xs = sc12.tile([64, TOK + 256], BF16, tag="xs")
                nc.gpsimd.memset(xs[:], 0.0)
                xs_img = xs[:, 0:TOK].rearrange(
                    "p (r c) -> p r c", r=SG)[:, 2:114, 2:114]
                nc.gpsimd.dma_start(out=xs_img, in_=x_in[:])
                xc_img = xcs[0:64, XC_OFF:XC_OFF + NG].rearrange(
                    "p (r c) -> p r c", r=CG)[:, 1:113, 1:113]
                nc.gpsimd.dma_start(out=xc_img, in_=x_in[:])
                xc_img2 = xcs[64:128, XC_OFF - CG:XC_OFF - CG + NG].rearrange(
                    "p (r c) -> p r c", r=CG)[:, 1:113, 1:113]
                nc.gpsimd.dma_start(out=xc_img2, in_=x_in[:])
                idt = sc12.tile([64, 64], BF16, tag="idt")
                nc.sync.dma_start(out=idt[:], in_=id64_c[:])
                qeng2 = (nc.sync, nc.scalar)
                TB = 4  # token blocks (128 tokens each) per copy+DMA
                nb = (TOK + 127) // 128  # 109
                for it, t0b in enumerate(range(0, nb, TB)):
                    nblk = min(TB, nb - t0b)
                    pst = ps12.tile([128, TB * 256], BF16, tag="pst")
                    for u in range(nblk):
                        j0 = (t0b + u) * 128
                        for di, dlt in enumerate((0, 1, SG, SG + 1)):
                            nc.tensor.transpose(
                                out=pst[:, u * 256 + di * 64:
                                        u * 256 + (di + 1) * 64],
                                in_=xs[:, j0 + dlt:j0 + dlt + 128],
                                identity=idt[:],
                            )
                    stg = stgp.tile([128, TB * 256], BF16, tag="stg")
                    if it % 2 == 0:
                        nc.scalar.copy(stg[:, 0:nblk * 256],
                                       pst[:, 0:nblk * 256])
                    else:
                        nc.vector.tensor_copy(stg[:, 0:nblk * 256],
                                              pst[:, 0:nblk * 256])
                    qeng2[it % 2].dma_start(
                        out=bass.AP(tok_dram, t0b * 128 * 256,
                                    [[256, 128], [128 * 256, nblk], [1, 256]]),
                        in_=stg[:, 0:nblk * 256])

            # ---- stages 3-5 ----
            # per-group idx tables: [128 part (8 replicas x 16 r), group, 576]
            gidx_w = big.tile([128, NGROUP, GIDX // 16], I16, tag="gidx_w")
            with (
                tc.tile_pool(name="sc34", bufs=1) as sc34,
                tc.tile_pool(name="ps34", bufs=2, space="PSUM") as ps34,
            ):
                scr = sc34.tile([128, 4 * GSTR], F32, tag="scr")
                wpT = sc34.tile([128, 3, 41], BF16, tag="wpT")
                wsT = sc34.tile([64, 3, 41], BF16, tag="wsT")
                boY = sc34.tile([128, 1], F32, tag="boY")
                boX = sc34.tile([128, 1], F32, tag="boX")
                bY = sc34.tile([128, 4 * GSTR], BF16, tag="bY")
                bX = sc34.tile([128, 4 * GSTR], BF16, tag="bX")
                mag1 = sc34.tile([128, 1], F32, tag="mag1")
                mag2 = sc34.tile([128, 1], F32, tag="mag2")
                nc.sync.dma_start(out=mag1[:], in_=mag1_c[:])
                nc.sync.dma_start(out=mag2[:], in_=mag2_c[:])
                nc.sync.dma_start(out=wpT[:], in_=wpair_c[:])
                nc.sync.dma_start(out=wsT[:], in_=wsing_c[:])
                nc.sync.dma_start(out=boY[:], in_=boffY_c[:])
                nc.sync.dma_start(out=boX[:], in_=boffX_c[:])
                nc.sync.dma_start(out=bY[:], in_=bY_c[:])
                nc.sync.dma_start(out=bX[:], in_=bX_c[:])
                for ch in range(NCHUNK):
                    n0 = ch * CHUNK
                    psc = ps34.tile([41, CHUNK], F32, tag="psc")
                    for p in range(3):  # taps (p, p+3) paired, K=128
                        dlt = -CG + (p - 1)
                        nc.tensor.matmul(
                            psc[:], wpT[:, p, :],
                            xcs[:, XC_OFF + dlt + n0:XC_OFF + dlt + n0 + CHUNK],
                            start=(p == 0), stop=False,
                        )
                    for q in range(3):  # taps 6,7,8 single, K=64
                        dlt = CG + (q - 1)
                        nc.tensor.matmul(
                            psc[:], wsT[:, q, :],
                            xcs[0:64, XC_OFF + dlt + n0:XC_OFF + dlt + n0 + CHUNK],
                            start=False, stop=(q == 2),
                        )
                    g, hf = ch // 2, ch % 2
                    rows = slice(PBLK * (g // 4), PBLK * (g // 4) + 9)
                    cols = slice((g % 4) * GSTR + hf * CHUNK,
                                 (g % 4) * GSTR + (hf + 1) * CHUNK)
                    nc.vector.scalar_tensor_tensor(
                        out=pyP[rows, cols], in0=psc[0:9, :], scalar=boY[rows],
                        in1=bY[rows, cols], op0=ADD, op1=ADD)
                    nc.vector.scalar_tensor_tensor(
                        out=pxP[rows, cols], in0=psc[32:41, :], scalar=boX[rows],
                        in1=bX[rows, cols], op0=ADD, op1=ADD)

                # indices + bilinear coefs (bY/bX freed into wy1/wx1 roles)
                y0b = sc34.tile([128, 4 * GSTR], BF16, tag="y0b")
                x0b = sc34.tile([128, 4 * GSTR], BF16, tag="x0b")
                wy1 = bY
                wx1 = bX
                nc.scalar.add(scr[:], pyP[:], mag1[:])
                nc.scalar.add(y0b[:], scr[:], mag2[:])
                nc.vector.tensor_tensor(wy1[:], pyP[:], y0b[:], SUB)
                nc.scalar.add(scr[:], pxP[:], mag1[:])
                nc.scalar.add(x0b[:], scr[:], mag2[:])
                nc.vector.tensor_tensor(wx1[:], pxP[:], x0b[:], SUB)
                nc.vector.tensor_scalar(y0b[:], y0b[:], 0.0, 115.0, MAXOP, MINOP)
                nc.vector.tensor_scalar(x0b[:], x0b[:], 0.0, 115.0, MAXOP, MINOP)
                gidx_f = pyP
                nc.vector.scalar_tensor_tensor(
                    out=gidx_f[:], in0=y0b[:], scalar=float(SG), in1=x0b[:],
                    op0=MULT, op1=ADD)
                # scr is dead from here on; reuse its storage for the i16 idx
                gidx_i = scr[:].bitcast(I16)[:, 0:4 * GSTR]
                nc.vector.tensor_copy(gidx_i, gidx_f[:])
                wy0 = y0b
                wx0 = x0b
                nc.vector.tensor_scalar(wy0[:], wy1[:], -1.0, 1.0, MULT, ADD)
                nc.vector.tensor_scalar(wx0[:], wx1[:], -1.0, 1.0, MULT, ADD)

                # corner products: q-order inputs -> slot(j)-order outputs.
                # TL/BL write their planes directly (junk in unused rows is
                # finite: py/px planes are zero there). TR/BR go through a
                # temp plane, then SBUF->SBUF DMA shifts rows 32b+k down to
                # 32b+16+k (engine ops can't start at partition 16, DMA can).
                cTt = sc34.tile([128, 4 * GSTR], BF16, tag="cTt")

                def jvF(t):
                    return t[:].rearrange("p (g c r) -> p g r c",
                                          g=4, c=64, r=16)

                def qvF(t):
                    return t[:].rearrange("p (g r c) -> p g r c",
                                          g=4, r=16, c=64)

                nc.vector.tensor_tensor(jvF(cT2), qvF(wy0), qvF(wx0), MULT)
                nc.vector.tensor_tensor(jvF(cTt), qvF(wy0), qvF(wx1), MULT)
                for b in range(4):
                    nc.scalar.dma_start(
                        out=cT2[PBLK * b + 16:PBLK * b + 25, :],
                        in_=cTt[PBLK * b:PBLK * b + 9, :])
                nc.vector.tensor_tensor(jvF(cB2), qvF(wy1), qvF(wx0), MULT)
                nc.vector.tensor_tensor(jvF(cTt), qvF(wy1), qvF(wx1), MULT)
                for b in range(4):
                    nc.scalar.dma_start(
                        out=cB2[PBLK * b + 16:PBLK * b + 25, :],
                        in_=cTt[PBLK * b:PBLK * b + 9, :])

                # idx staging: SBUF rows (q-order) -> gl2[k][g][q], then a
                # DRAM->DRAM shuffle per tap into gl3[g][r][k][c]
                GB = 16 * KK * 64  # 9216 elements per group block
                qeng = (nc.sync, nc.scalar)
                for k in range(KK):
                    for b in range(4):
                        row = PBLK * b + k
                        qeng[(k * 4 + b) % 2].dma_start(
                            out=bass.AP(gl2_dram, (k * 16 + 4 * b) * GSTR,
                                        [[1, 4 * GSTR]]),
                            in_=gidx_i[row:row + 1, :],
                        )
                for k in range(KK):
                    qeng[k % 2].dma_start(
                        out=bass.AP(gl_dram, k * 64,
                                    [[GB, 16], [KK * 64, 16], [1, 64]]),
                        in_=bass.AP(gl2_dram, k * 16 * GSTR,
                                    [[GSTR, 16], [1, GSTR]]),
                    )
                # idx tables: gidx_w[16a+r, s, 64k+c] = gl3[s][r][k][c], i.e.
                # slot j = 1024k + 16c + r samples position q = 64r + c.
                # One batched read per replica block a (all 13 groups).
                for a in range(8):
                    qeng[a % 2].dma_start(
                        out=gidx_w[16 * a:16 * (a + 1), :, :],
                        in_=bass.AP(gl_dram, 0,
                                    [[KK * 64, 16], [GB, NGROUP], [1, KK * 64]]),
                    )

            sc15_cm.__exit__(None, None, None)

            # ---- stage 6: deformable conv main loop ----
            ydef = big.tile([C_OUT, N], BF16, tag="ydef")
            bd = big.tile([C_OUT, 1], F32, tag="bd")
            nc.sync.dma_start(out=bd[:], in_=bdcn_c[:])
            wdupS = big.tile([128, KK, 128], BF16, tag="wdupS")
            nc.sync.dma_start(out=wdupS[:], in_=wdup_c[:])
            sel2 = big.tile([128, 36, 128], BF16, tag="sel2")
            nc.sync.dma_start(out=sel2[:], in_=sel_c[:])

            sc6_cm = tc.tile_pool(name="sc6", bufs=2)
            sc6 = sc6_cm.__enter__()
            ps6_cm = tc.tile_pool(name="ps6", bufs=2, space="PSUM")
            ps6 = ps6_cm.__enter__()
            for s in range(NGROUP):
                # two half-gathers: one 9216-idx gather overflows the SWDGE
                # ring. 4608 = 4.5 taps, and every (k, cc) 512-chunk lies
                # wholly in one half.
                hh = GIDX // 2
                gtA = sc6.tile([128, 2, hh], BF16, tag="gtA")
                gtB = sc6.tile([128, 2, hh], BF16, tag="gtB")
                gts = [gtA, gtB]
                single_packet_flag = gather_sp
                for gh in range(2):
                    if no_gather:
                        pass  # timing ablation: downstream reads junk SBUF
                    elif gather_notr:
                        # timing ablation only: wrong output layout
                        nc.gpsimd.dma_gather(
                            out_ap=gts[gh][:].rearrange("p a n -> p (a n)")
                            .rearrange("p (n e) -> p n e", e=256),
                            in_ap=tok_dram[:],
                            idxs_ap=gidx_w[:, s, gh * (hh // 16):(gh + 1) * (hh // 16)],
                            num_idxs=hh, num_idxs_reg=hh, elem_size=256,
                            transpose=False, single_packet=False,
                        )
                    else:
                        nc.gpsimd.dma_gather(
                            out_ap=gts[gh][:], in_ap=tok_dram[:],
                            idxs_ap=gidx_w[:, s, gh * (hh // 16):(gh + 1) * (hh // 16)],
                            num_idxs=hh, num_idxs_reg=hh, elem_size=256,
                            transpose=True, single_packet=single_packet_flag,
                        )
                psyA = ppy.tile([C_OUT, CHUNK], F32, tag="psyA")
                psyB = ppy.tile([C_OUT, CHUNK], F32, tag="psyB")
                for k in range(KK):
                    for cc in range(GSLICE // CHUNK):  # 2
                        selk = sel2[:, (s // 4) * 9 + k, :]
                        cols = slice((s % 4) * GSTR + cc * CHUNK,
                                     (s % 4) * GSTR + (cc + 1) * CHUNK)
                        ctT = ps6.tile([128, CHUNK], F32, tag="ctT")
                        ctB = ps6.tile([128, CHUNK], F32, tag="ctB")
                        nc.tensor.matmul(ctT[:], selk,
                                         cT2[:, cols], start=True, stop=True)
                        nc.tensor.matmul(ctB[:], selk,
                                         cB2[:, cols], start=True, stop=True)
                        gT = work.tile([128, CHUNK], BF16, tag="gT")
                        gB = work.tile([128, CHUNK], BF16, tag="gB")
                        col = k * GSLICE + cc * CHUNK
                        gt = gts[col // hh]
                        gsl = slice(col % hh, col % hh + CHUNK)
                        if cc == 1 and act_stage:
                            # ACT-staged bf16 coefs -> DVE runs in 2x mode
                            stT = work.tile([128, CHUNK], BF16, tag="stT")
                            stB = work.tile([128, CHUNK], BF16, tag="stB")
                            nc.scalar.add(stT[:], ctT[:], 0.0)
                            nc.scalar.add(stB[:], ctB[:], 0.0)
                            nc.vector.tensor_tensor(
                                gT[:], gt[:, 0, gsl], stT[:], MULT)
                            nc.vector.tensor_tensor(
                                gB[:], gt[:, 1, gsl], stB[:], MULT)
                        else:
                            nc.vector.tensor_tensor(
                                gT[:], gt[:, 0, gsl], ctT[:], MULT)
                            nc.vector.tensor_tensor(
                                gB[:], gt[:, 1, gsl], ctB[:], MULT)
                        psy = psyA if cc == 0 else psyB
                        nc.tensor.matmul(psy[:], wdupS[:, k, :], gT[:],
                                         start=(k == 0), stop=False,
                                         skip_group_check=True)
                        nc.tensor.matmul(psy[:], wdupS[:, k, :], gB[:],
                                         start=False, stop=(k == KK - 1),
                                         skip_group_check=True)
                for cc in range(GSLICE // CHUNK):
                    psy = psyA if cc == 0 else psyB
                    # un-permute: psy col 16c+r -> ydef col 64r+c (+32cc, +1024s)
                    yv_blk = ydef[:, s * GSLICE:(s + 1) * GSLICE].rearrange(
                        "p (r c) -> p c r", r=16, c=64)[:, 32 * cc:32 * (cc + 1), :]
                    # Identity everywhere in stage 6: one ACT function, no
                    # act-table reloads between staging copies and this add
                    nc.scalar.add(
                        yv_blk,
                        psy[:].rearrange("p (c r) -> p c r", c=32, r=16),
                        bd[:])
            ps6_cm.__exit__(None, None, None)
            sc6_cm.__exit__(None, None, None)

            # ---- stage 7: BN stats + AllReduce ----
            ssum = big.tile([C_OUT, 8], F32, tag="ssum")
            ssq = big.tile([C_OUT, 8], F32, tag="ssq")
            yv = ydef[:, 0:NG].rearrange("p (r c) -> p r c", r=CG)
            for r in range(7):
                vap = yv[:, 1 + r * 16:1 + (r + 1) * 16, 1:113]
                nc.vector.tensor_reduce(
                    ssum[:, r:r + 1], vap, axis=mybir.AxisListType.XY, op=ADD)
                sqscr = work.tile([C_OUT, 16 * W], F32, tag="ofin")
                nc.vector.scalar_tensor_tensor(
                    out=sqscr[:].rearrange("p (a b) -> p a b", a=16), in0=vap,
                    scalar=1.0, in1=vap, op0=MULT, op1=MULT,
                    accum_out=ssq[:, r:r + 1])
            st2 = big.tile([C_OUT, 2], F32, tag="st2")
            nc.vector.tensor_reduce(
                st2[:, 0:1], ssum[:, 0:7], axis=mybir.AxisListType.X, op=ADD)
            nc.vector.tensor_reduce(
                st2[:, 1:2], ssq[:, 0:7], axis=mybir.AxisListType.X, op=ADD)
            nc.sync.dma_start(out=stats_in[:], in_=st2[:])
            if no_cc:
                nc.sync.dma_start(out=stats_out[:], in_=stats_in[:])
            else:
                nc.gpsimd.collective_compute(
                    "AllReduce", ADD, replica_groups=[list(range(n_cores))],
                    ins=[stats_in[:]], outs=[stats_out[:]])
            stg2 = big.tile([C_OUT, 2], F32, tag="stg2")
            nc.sync.dma_start(out=stg2[:], in_=stats_out[:])

            # ---- stage 8: BN affine + ReLU + store ----
            gam = big.tile([C_OUT, 1], F32, tag="gamt")
            bet = big.tile([C_OUT, 1], F32, tag="bett")
            nc.sync.dma_start(out=gam[:], in_=gam_c[:])
            nc.sync.dma_start(out=bet[:], in_=bet_c[:])
            NTOT = float(n_cores * H * W)
            mean = big.tile([C_OUT, 1], F32, tag="mean")
            var = big.tile([C_OUT, 1], F32, tag="var")
            nc.vector.tensor_scalar(mean[:], stg2[:, 0:1], 1.0 / NTOT, None, MULT)
            nc.vector.tensor_scalar(var[:], stg2[:, 1:2], 1.0 / NTOT, None, MULT)
            m2 = big.tile([C_OUT, 1], F32, tag="m2")
            nc.vector.tensor_tensor(m2[:], mean[:], mean[:], MULT)
            nc.vector.tensor_tensor(var[:], var[:], m2[:], SUB)
            nc.vector.tensor_scalar(var[:], var[:], EPS, None, ADD)
            sd = big.tile([C_OUT, 1], F32, tag="sd")
            nc.scalar.activation(sd[:], var[:], mybir.ActivationFunctionType.Sqrt)
            rsd = big.tile([C_OUT, 1], F32, tag="rsd")
            nc.vector.reciprocal(rsd[:], sd[:])
            aa = big.tile([C_OUT, 1], F32, tag="aa")
            bb2 = big.tile([C_OUT, 1], F32, tag="bb2")
            nc.vector.tensor_tensor(aa[:], gam[:], rsd[:], MULT)
            nc.vector.tensor_tensor(bb2[:], aa[:], mean[:], MULT)
            nc.vector.tensor_tensor(bb2[:], bet[:], bb2[:], SUB)
            for r in range(7):
                vap = yv[:, 1 + r * 16:1 + (r + 1) * 16, 1:113]
                ofin = work.tile([C_OUT, 16 * W], F32, tag="ofin")
                nc.scalar.activation(
                    ofin[:].rearrange("p (a b) -> p a b", a=16), vap,
                    mybir.ActivationFunctionType.Relu, bias=bb2[:], scale=aa[:])
                nc.sync.dma_start(
                    out=y_out[:, r * 16:(r + 1) * 16, :],
                    in_=ofin[:].rearrange("p (a b) -> p a b", a=16))

            if loop is not None:
                loop.__exit__(None, None, None)

    nc.compile()
    return nc


def kernel(x, w_off, b_off, w_dcn, b_dcn, gamma, beta):
    x = np.asarray(x, np.float32)
    nc = build_nc(
        np.asarray(w_off, np.float32), np.asarray(b_off, np.float32),
        np.asarray(w_dcn, np.float32), np.asarray(b_dcn, np.float32),
        np.asarray(gamma, np.float32), np.asarray(beta, np.float32),
    )
    in_maps = [{"x": np.ascontiguousarray(x[b])} for b in range(8)]
    res = run_bass_kernel_spmd(nc, in_maps, list(range(8)))
    return np.stack([res.results[b]["y"] for b in range(8)], 0).astype(np.float32)
